# revision 6
# baseline (speedup 1.0000x reference)
"""Batched 3-layer GCN (nn_BatchGCN) on 8 TRN2 NeuronCores — one graph per core.

Math per graph, per layer:  h' = Ahat @ (h @ W.T + b),  Ahat = D^-1/2 A D^-1/2
(duplicate edges accumulate; relu between layers 1-2).  The symmetric
normalization factors node-wise:
    out[i] = dinv[i] * sum_{e: row_e==i} (dinv * z)[col_e],  z = h@W.T + b
so no per-edge scalar work is needed — only dense node-wise scaling.

Device algorithm (per core; no per-edge DMA scatter — scatter is done with
TensorEngine one-hot matmuls, which is deterministic and avoids the
duplicate-index hazard of DMA scatter-add):
  - host buckets the edge list by destination block (128 nodes), CB chunks of
    128 edge slots per block; dummy slots carry row-offset 999 so their
    one-hot column is all-zero and they contribute nothing
  - degree pass: per block, one-hot [128e x 128d] matmuls against an all-ones
    rhs accumulate deg (replicated across the 64 feature cols) in PSUM;
    dinv = (deg>0)/sqrt(max(deg,1))
  - per layer: z^T = W_T.T @ h^T on PE (feat-major, packed two node-halves on
    128 partitions), +bias on DVE, PE-transpose to node-major, dinv scaling
    fused into the ScalarEngine copy (scale=), one contiguous DMA of z~ to a
    partition-major DRAM layout; dma_gather fetches the 256B messages per
    4-block chunk (the only indexed DMA; gathers are race-free); DVE builds
    the one-hot for each block with a single broadcast-AP is_equal; CB
    matmuls accumulate the block's segment sum in PSUM; relu*dinv is fused
    into the ScalarEngine activation that drains PSUM.
All compute is f32; results match the f32 reference to ~1e-5 relative.

Host-side work is index/layout marshaling only (transpose/pad/bucket the
given arrays); all arithmetic on tensor values happens on-device.
"""
from dataclasses import dataclass

import numpy as np

import concourse.bacc as bacc
import concourse.mybir as mybir
import concourse.tile as tile
from concourse.bass import broadcast_tensor_aps
from concourse.bass_utils import run_bass_kernel_spmd
from concourse.library_config import mlp

B, NV, E, F = 8, 10000, 160000, 64
N = 10240          # padded node count (80 blocks of 128)
CORES = list(range(8))


@dataclass(frozen=True)
class _Cfg:
    N: int
    NV: int
    F: int
    CB: int        # chunks (of 128 edge slots) per destination block
    GB: int        # blocks per gather chunk
    layers: int = 3

    @property
    def nblk(self):
        return self.N // 128

    @property
    def epad(self):
        return self.nblk * self.CB * 128

    @property
    def ngch(self):
        return self.nblk // self.GB


def _build(cfg: _Cfg, trips: int = 1):
    N, F, CB = cfg.N, cfg.F, cfg.CB
    NBLK, EPAD, GB, NGCH = cfg.nblk, cfg.epad, cfg.GB, cfg.ngch
    NQ = N // 512

    nc = bacc.Bacc("TRN2", debug=False, num_swdge_queues=4)
    x_hbm = nc.dram_tensor("x_packed", [128, N // 2], mybir.dt.float32, kind="ExternalInput")
    w_hbm = nc.dram_tensor("w_t", [128, cfg.layers * F], mybir.dt.float32, kind="ExternalInput")
    b_hbm = nc.dram_tensor("bias", [128, cfg.layers], mybir.dt.float32, kind="ExternalInput")
    i_hbm = nc.dram_tensor("ident", [128, 128], mybir.dt.float32, kind="ExternalInput")
    t_hbm = nc.dram_tensor("iota_t", [128, CB * 128], mybir.dt.float32, kind="ExternalInput")
    r_hbm = nc.dram_tensor("rowoff", [128, EPAD // 128], mybir.dt.float32, kind="ExternalInput")
    c_hbm = nc.dram_tensor("colr", [128, EPAD // 16], mybir.dt.int16, kind="ExternalInput")
    out_hbm = nc.dram_tensor("out_pm", [128, NBLK * F], mybir.dt.float32, kind="ExternalOutput")
    zdram = nc.dram_tensor("zdram", [N, F], mybir.dt.float32)

    with tile.TileContext(nc) as tc:
        with (
            tc.tile_pool(name="const", bufs=1) as cp,
            tc.tile_pool(name="state", bufs=1) as sp,
            tc.tile_pool(name="oh", bufs=3) as ohp,
            tc.tile_pool(name="msg", bufs=3) as mp,
            tc.tile_pool(name="zb", bufs=2) as zp,
            tc.tile_pool(name="pz", bufs=1, space="PSUM") as pz,
            tc.tile_pool(name="pt", bufs=2, space="PSUM") as pt,
            tc.tile_pool(name="psc", bufs=4, space="PSUM") as psc,
            tc.tile_pool(name="pbt", bufs=1, space="PSUM") as pbt,
        ):
            nc.gpsimd.load_library(mlp)

            wt = cp.tile([128, cfg.layers, F], mybir.dt.float32)
            nc.sync.dma_start(wt[:], w_hbm[:].rearrange("p (l f) -> p l f", l=cfg.layers))
            bs = cp.tile([128, cfg.layers], mybir.dt.float32)
            nc.sync.dma_start(bs[:], b_hbm[:])
            ident = cp.tile([128, 128], mybir.dt.float32)
            nc.sync.dma_start(ident[:], i_hbm[:])
            iota = cp.tile([128, CB * 128], mybir.dt.float32)
            nc.sync.dma_start(iota[:], t_hbm[:])
            rowoff = cp.tile([128, EPAD // 128], mybir.dt.float32)
            nc.sync.dma_start(rowoff[:], r_hbm[:])
            colr = cp.tile([128, EPAD // 16], mybir.dt.int16)
            nc.sync.dma_start(colr[:], c_hbm[:])
            ones = cp.tile([128, F], mybir.dt.bfloat16)
            nc.vector.memset(ones[:], 1.0)

            hA = sp.tile([128, N // 2], mybir.dt.float32, tag="hA")
            hB = sp.tile([128, N // 2], mybir.dt.float32, tag="hB")
            dinv = sp.tile([128, NBLK, F], mybir.dt.float32, tag="dinv")
            stage = sp.tile([128, NBLK, F], mybir.dt.float32, tag="stage")
            # hB doubles as the transient deg>0 mask (consumed before any h write)
            mask = hB[:].rearrange("p (c f) -> p c f", c=NBLK)

            def onehot_for(b, dt=mybir.dt.float32):
                # oh[e, c*128+d] = (rowoff[e, b*CB+c] == d)
                oh = ohp.tile([128, CB * 128], dt, tag="oh")
                ro3 = rowoff[:, b * CB:(b + 1) * CB].rearrange("p c -> p c ()")
                io3 = iota[:].rearrange("p (c d) -> p c d", c=CB)
                a, bb = broadcast_tensor_aps(io3, ro3)
                nc.vector.tensor_tensor(
                    out=oh[:].rearrange("p (c d) -> p c d", c=CB),
                    in0=a, in1=bb, op=mybir.AluOpType.is_equal)
                return oh

            def seg_matmuls(ps, oh, rhs_fn):
                for c in range(CB):
                    nc.tensor.matmul(
                        ps[:], oh[:, c * 128:(c + 1) * 128], rhs_fn(c),
                        start=(c == 0), stop=(c == CB - 1))

            def one_trip():
                nc.sync.dma_start(hA[:], x_hbm[:])
                # ---- degree pass ----
                for b in range(NBLK):
                    ps = psc.tile([128, F], mybir.dt.float32, tag="psc")
                    oh = onehot_for(b, mybir.dt.bfloat16)
                    seg_matmuls(ps, oh, lambda c: ones[:])
                    nc.scalar.copy(stage[:, b], ps[:])

                nc.vector.tensor_scalar(out=mask, in0=stage[:], scalar1=0.5,
                                        scalar2=None, op0=mybir.AluOpType.is_gt)
                nc.vector.tensor_scalar(out=stage[:], in0=stage[:], scalar1=1.0,
                                        scalar2=None, op0=mybir.AluOpType.max)
                nc.scalar.activation(stage[:], stage[:], mybir.ActivationFunctionType.Sqrt)
                nc.vector.reciprocal(stage[:], stage[:])
                nc.vector.tensor_tensor(out=dinv[:], in0=stage[:], in1=mask,
                                        op=mybir.AluOpType.mult)

                hcur = hA
                for lay in range(cfg.layers):
                    hnxt = hB if hcur is hA else hA
                    for q in range(NQ):
                        half = 0 if q < NQ // 2 else 64
                        qq = q % (NQ // 2)
                        pzt = pz.tile([64, 512], mybir.dt.float32, tag="pz")
                        nc.tensor.matmul(
                            pzt[:], wt[half:half + 64, lay],
                            hcur[half:half + 64, qq * 512:(qq + 1) * 512],
                            start=True, stop=True)
                        zb = zp.tile([64, 512], mybir.dt.float32, tag="zb")
                        nc.vector.tensor_scalar(
                            out=zb[:], in0=pzt[:],
                            scalar1=bs[half:half + 64, lay:lay + 1],
                            scalar2=None, op0=mybir.AluOpType.add)
                        for j in range(4):
                            blk = 4 * q + j
                            ptt = pt.tile([128, F], mybir.dt.float32, tag="pt")
                            nc.tensor.transpose(
                                ptt[:], zb[:, j * 128:(j + 1) * 128], ident[:64, :64])
                            nc.scalar.activation(
                                stage[:, blk], ptt[:],
                                mybir.ActivationFunctionType.Copy,
                                scale=dinv[:, blk, 0:1])
                    nc.sync.dma_start(
                        zdram[:].rearrange("(p c) f -> p c f", p=128), stage[:])
                    for g in range(NGCH):
                        nidx = GB * CB * 128
                        msgs = mp.tile([128, GB * CB, F], mybir.dt.float32, tag="msgs")
                        nc.gpsimd.dma_gather(
                            msgs[:], zdram[:],
                            colr[:, g * (nidx // 16):(g + 1) * (nidx // 16)],
                            nidx, nidx, F, single_packet=False,
                            queue_num=g % 4)
                        for bb in range(GB):
                            b = g * GB + bb
                            ps = psc.tile([128, F], mybir.dt.float32, tag="psc")
                            oh = onehot_for(b)
                            seg_matmuls(ps, oh, lambda c, _bb=bb: msgs[:, _bb * CB + c])
                            if lay < cfg.layers - 1:
                                hm = zp.tile([128, F], mybir.dt.float32, tag="hm")
                                nc.scalar.activation(
                                    hm[:], ps[:], mybir.ActivationFunctionType.Relu,
                                    scale=dinv[:, b, 0:1])
                                pbtt = pbt.tile([64, 128], mybir.dt.float32, tag="pbt")
                                nc.tensor.transpose(pbtt[:], hm[:], ident[:])
                                half = 0 if b < NBLK // 2 else 64
                                bq = b % (NBLK // 2)
                                nc.scalar.copy(
                                    hnxt[half:half + 64, bq * 128:(bq + 1) * 128], pbtt[:])
                            else:
                                nc.scalar.activation(
                                    stage[:, b], ps[:],
                                    mybir.ActivationFunctionType.Copy,
                                    scale=dinv[:, b, 0:1])
                    hcur = hnxt
                nc.sync.dma_start(
                    out_hbm[:].rearrange("p (c f) -> p c f", c=NBLK), stage[:])

            for _trip in range(trips):
                one_trip()

    nc.compile()
    return nc


def _prep_inputs(cfg: _Cfg, x, edge_index, Ws, bs_):
    """Index/layout marshaling for one graph (no value arithmetic)."""
    N, F, CB, NV = cfg.N, cfg.F, cfg.CB, cfg.NV
    NBLK, EPAD = cfg.nblk, cfg.epad
    row = np.asarray(edge_index[:, 0], np.int64)
    col = np.asarray(edge_index[:, 1], np.int64)
    blk = row >> 7
    order = np.argsort(blk, kind="stable")
    counts = np.bincount(blk, minlength=NBLK)
    assert counts.max() <= CB * 128, f"block overflow: {counts.max()}"
    starts = np.cumsum(counts) - counts
    base = np.repeat(np.arange(NBLK) * CB * 128, counts)
    within = np.arange(len(row)) - np.repeat(starts, counts)
    slots = base + within
    rowoff = np.full(EPAD, 999.0, np.float32)
    colv = np.zeros(EPAD, np.int64)
    rowoff[slots] = (row & 127)[order]
    colv[slots] = col[order]
    # remap node id -> row of the partition-major z~ DRAM layout
    colr = ((colv & 127) * NBLK + (colv >> 7)).astype(np.int16)

    def wrap16(a):
        w = a.reshape(-1, 16).T
        return np.tile(w, (8, 1))

    rowoff_t = np.ascontiguousarray(rowoff.reshape(-1, 128).T)
    colr_t = wrap16(colr)

    xT = np.zeros((64, N), np.float32)
    xT[:, :NV] = np.asarray(x, np.float32).T
    x_packed = np.concatenate([xT[:, :N // 2], xT[:, N // 2:]], axis=0)

    w_t = np.zeros((128, len(Ws), F), np.float32)
    bias = np.zeros((128, len(Ws)), np.float32)
    for l, (W, b) in enumerate(zip(Ws, bs_)):
        w_t[:64, l] = np.asarray(W, np.float32).T
        w_t[64:, l] = np.asarray(W, np.float32).T
        bias[:64, l] = np.asarray(b, np.float32)
        bias[64:, l] = np.asarray(b, np.float32)

    return {
        "x_packed": x_packed,
        "w_t": np.ascontiguousarray(w_t.reshape(128, -1)),
        "bias": bias,
        "ident": np.eye(128, dtype=np.float32),
        "iota_t": np.tile(np.tile(np.arange(128, dtype=np.float32), CB), (128, 1)),
        "rowoff": rowoff_t,
        "colr": colr_t,
    }


def _unpack_output(cfg: _Cfg, out_pm):
    o = out_pm.reshape(128, cfg.nblk, cfg.F).transpose(1, 0, 2).reshape(cfg.N, cfg.F)
    return o[:cfg.NV]


def kernel(x, edge_index, W1, b1, W2, b2, W3, b3):
    x = np.asarray(x)
    edge_index = np.asarray(edge_index)
    Ws = [np.asarray(W1), np.asarray(W2), np.asarray(W3)]
    bs_ = [np.asarray(b1), np.asarray(b2), np.asarray(b3)]
    nb = x.shape[0]
    assert x.shape == (B, NV, F) and edge_index.shape == (B, E, 2)

    # destination-block budget (recompiles only for pathological inputs)
    maxcnt = max(
        int(np.bincount(np.asarray(edge_index[g, :, 0], np.int64) >> 7,
                        minlength=N // 128).max())
        for g in range(nb)
    )
    CB = max(16, -(-maxcnt // 128))
    cfg = _Cfg(N=N, NV=NV, F=F, CB=CB, GB=2)

    in_maps = [_prep_inputs(cfg, x[g], edge_index[g], Ws, bs_) for g in range(nb)]
    nc = _build(cfg)
    try:
        res = run_bass_kernel_spmd(nc, in_maps, CORES).results
    except Exception:
        # transient NRT device wedge recovers on a fresh attempt
        res = run_bass_kernel_spmd(nc, in_maps, CORES).results
    out = np.stack([_unpack_output(cfg, res[g]["out_pm"]) for g in range(nb)])
    return out.astype(np.float32)



# revision 7
# speedup vs baseline: 3.4088x; 3.4088x over previous
"""Batched 3-layer GCN (nn_BatchGCN) on 8 TRN2 NeuronCores — one graph per core.

Math per graph, per layer:  h' = Ahat @ (h @ W.T + b),  Ahat = D^-1/2 A D^-1/2
(duplicate edges accumulate; relu between layers 1-2).  The symmetric
normalization factors node-wise:
    out[i] = dinv[i] * sum_{e: row_e==i} (dinv * z)[col_e],  z = h@W.T + b
so no per-edge scalar work is needed — only dense node-wise scaling.

Cell-grid device algorithm (per core).  The edge permutation (gather of
z[col] into dest-bucket order) is done as an 80x80 (src-block x dest-block)
grid transpose through DRAM with contiguous-cell DMAs — no per-edge DMA
descriptors (the old dma_gather path was descriptor-count-bound at ~10ns per
256B descriptor, 160K descriptors/layer):
  - host buckets edges into (s,d) cells of 32 slots; overflow edges (~2%)
    fall back to the per-edge dma_gather path (OV_CB chunks per dest block)
  - per layer: z~ = dinv*(h@W.T+b) node-major (PE z-matmuls + transposes,
    scale fused into the ScalarE drain), fp16 copy for the gather matmuls
  - GATHER per src block sb: host-shipped offset-major one-hot OHT[off,slot]
    (fp16, pure index data) as PE stationary x z~[sb] produces slot-major
    messages; drained fp16; one contiguous DMA writes sb's cell row
  - SCATTER per dest block d: one strided DMA reads d's 80 cells (4KB/cell,
    one per SBUF partition); is_equal one-hot [s, doff, k] built on DVE with
    packed-fp16 operands (2x fast path) + 32 accumulating matmuls contract
    over the 80 cell partitions; overflow chunk matmuls add in; relu*dinv
    fused into the PSUM drain
  - degree pass reuses the same one-hot structure against a ones rhs
Messages travel fp16 (~1e-4 rel err vs f32 reference; tolerance 2e-2).

Host-side work is index/layout marshaling only (transpose/pad/bucket the
given index arrays and expand index one-hots); all arithmetic on tensor
values happens on-device.
"""
from dataclasses import dataclass

import numpy as np

import concourse.bacc as bacc
import concourse.mybir as mybir
import concourse.tile as tile
from concourse.bass import broadcast_tensor_aps
from concourse.bass_utils import run_bass_kernel_spmd
from concourse.library_config import mlp

B, NV, E, F = 8, 10000, 160000, 64
N = 10240
NBLK = 80
CELL = 32
SROW = NBLK * CELL          # 2560 slots per src-block row (= per dest block)
SCH = SROW // 128           # 20 gather-matmul chunks per src block
EPADG = NBLK * SROW         # 204800 grid slots
CORES = list(range(8))


@dataclass(frozen=True)
class Cfg:
    OV_CB: int              # overflow chunks (of 128 slots) per dest block
    OVG: int = 8            # dest blocks per overflow gather call
    layers: int = 3

    @property
    def epadov(self):
        return NBLK * self.OV_CB * 128


def make_cfg(edge_index):
    mx = 0
    for g in range(edge_index.shape[0]):
        row = np.asarray(edge_index[g, :, 0], np.int64)
        col = np.asarray(edge_index[g, :, 1], np.int64)
        cell = (col >> 7) * NBLK + (row >> 7)
        ccnt = np.bincount(cell, minlength=NBLK * NBLK)
        ovper = np.maximum(ccnt - CELL, 0).reshape(NBLK, NBLK).sum(axis=0)
        mx = max(mx, int(ovper.max()))
    return Cfg(OV_CB=max(1, -(-mx // 128)))


def _build(cfg: Cfg, trips: int = 1):
    OV_CB = cfg.OV_CB
    EPADOV = cfg.epadov
    OVG = cfg.OVG
    NQ = N // 512

    nc = bacc.Bacc("TRN2", debug=False)
    x_hbm = nc.dram_tensor("x_packed", [128, N // 2], mybir.dt.float32, kind="ExternalInput")
    w_hbm = nc.dram_tensor("w_t", [128, cfg.layers * F], mybir.dt.float32, kind="ExternalInput")
    b_hbm = nc.dram_tensor("bias", [128, cfg.layers], mybir.dt.float32, kind="ExternalInput")
    i_hbm = nc.dram_tensor("ident", [128, 128], mybir.dt.float32, kind="ExternalInput")
    t_hbm = nc.dram_tensor("iota_t", [128, SROW], mybir.dt.float16, kind="ExternalInput")
    tx_hbm = nc.dram_tensor("iota_x", [128, 128 * CELL], mybir.dt.float16, kind="ExternalInput")
    rc_hbm = nc.dram_tensor("rowcell", [128, SROW], mybir.dt.float16, kind="ExternalInput")
    og_hbm = nc.dram_tensor("ohg", [NBLK * 128, SROW], mybir.dt.float16, kind="ExternalInput")
    ovr_hbm = nc.dram_tensor("ovrow", [128, EPADOV // 128], mybir.dt.float16, kind="ExternalInput")
    ovc_hbm = nc.dram_tensor("ovcolr", [128, EPADOV // 16], mybir.dt.int16, kind="ExternalInput")
    out_hbm = nc.dram_tensor("out_pm", [128, NBLK * F], mybir.dt.float32, kind="ExternalOutput")
    zdram = nc.dram_tensor("zdram", [N, F], mybir.dt.float32)
    cells = nc.dram_tensor("cells", [EPADG, F], mybir.dt.float16)

    with tile.TileContext(nc) as tc:
        with (
            tc.tile_pool(name="const", bufs=1) as cp,
            tc.tile_pool(name="state", bufs=1) as sp,
            tc.tile_pool(name="ohgb", bufs=2) as obp,
            tc.tile_pool(name="oh", bufs=3) as ohp,
            tc.tile_pool(name="mst", bufs=2) as msp,
            tc.tile_pool(name="msg", bufs=3) as mp,
            tc.tile_pool(name="ovm", bufs=2) as ovp,
            tc.tile_pool(name="zb", bufs=2) as zp,
            tc.tile_pool(name="pz", bufs=1, space="PSUM") as pz,
            tc.tile_pool(name="pt", bufs=2, space="PSUM") as pt,
            tc.tile_pool(name="pmg", bufs=2, space="PSUM") as pmg,
            tc.tile_pool(name="psc", bufs=2, space="PSUM") as psc,
            tc.tile_pool(name="pbt", bufs=1, space="PSUM") as pbt,
        ):
            nc.gpsimd.load_library(mlp)

            wt = cp.tile([128, cfg.layers, F], mybir.dt.float32)
            nc.sync.dma_start(wt[:], w_hbm[:].rearrange("p (l f) -> p l f", l=cfg.layers))
            bs = cp.tile([128, cfg.layers], mybir.dt.float32)
            nc.sync.dma_start(bs[:], b_hbm[:])
            ident = cp.tile([128, 128], mybir.dt.float32)
            nc.sync.dma_start(ident[:], i_hbm[:])
            iota = cp.tile([128, SROW], mybir.dt.float16)
            nc.sync.dma_start(iota[:], t_hbm[:])
            iotax = cp.tile([128, 128 * CELL], mybir.dt.float16)
            nc.sync.dma_start(iotax[:], tx_hbm[:])
            rowcell = cp.tile([128, SROW], mybir.dt.float16)
            nc.sync.dma_start(rowcell[:], rc_hbm[:])
            ovrow = cp.tile([128, EPADOV // 128], mybir.dt.float16)
            nc.sync.dma_start(ovrow[:], ovr_hbm[:])
            ovcolr = cp.tile([128, EPADOV // 16], mybir.dt.int16)
            nc.sync.dma_start(ovcolr[:], ovc_hbm[:])
            ones_b = cp.tile([128, F], mybir.dt.float16)
            nc.vector.memset(ones_b[:], 1.0)
            ones_f = cp.tile([128, F], mybir.dt.float32)
            nc.vector.memset(ones_f[:], 1.0)

            hA = sp.tile([128, N // 2], mybir.dt.float32, tag="hA")
            hB = sp.tile([128, N // 2], mybir.dt.float32, tag="hB")
            dinv = sp.tile([128, NBLK, F], mybir.dt.float32, tag="dinv")
            stage = sp.tile([128, NBLK, F], mybir.dt.float32, tag="stage")
            zb16 = sp.tile([128, NBLK, F], mybir.dt.float16, tag="zb16")
            mask = hB[:].rearrange("p (c f) -> p c f", c=NBLK)

            def onehot_cell(d):
                # cell-major scatter one-hot, [s, doff, k] layout: every DVE
                # operand keeps a packed 2-byte last dim (2x fast path)
                oh = ohp.tile([80, 128, CELL], mybir.dt.float16, tag="oh")
                ro3 = rowcell[:80, d * CELL:(d + 1) * CELL].rearrange("p k -> p () k")
                io3 = iotax[:80].rearrange("p (j k) -> p j k", j=128)
                a, bb = broadcast_tensor_aps(io3, ro3)
                nc.vector.tensor_tensor(out=oh[:], in0=a, in1=bb,
                                        op=mybir.AluOpType.is_equal)
                return oh

            def onehot_ov(d):
                oh = ohp.tile([128, OV_CB * 128], mybir.dt.float32, tag="ohov")
                ro3 = ovrow[:, d * OV_CB:(d + 1) * OV_CB].rearrange("p c -> p c ()")
                io3 = iota[:, :OV_CB * 128].rearrange("p (c k) -> p c k", c=OV_CB)
                a, bb = broadcast_tensor_aps(io3, ro3)
                nc.vector.tensor_tensor(
                    out=oh[:].rearrange("p (c k) -> p c k", c=OV_CB),
                    in0=a, in1=bb, op=mybir.AluOpType.is_equal)
                return oh

            def one_trip():
                nc.sync.dma_start(hA[:], x_hbm[:])
                # ---- degree pass ----
                for d in range(NBLK):
                    ps = psc.tile([128, F], mybir.dt.float32, tag="psc")
                    oh = onehot_cell(d)
                    for k in range(CELL):
                        nc.tensor.matmul(ps[:], oh[:, :, k], ones_b[:80],
                                         start=(k == 0), stop=False)
                    ohov = onehot_ov(d)
                    for c in range(OV_CB):
                        nc.tensor.matmul(ps[:], ohov[:, c * 128:(c + 1) * 128],
                                         ones_f[:], start=False, stop=(c == OV_CB - 1))
                    nc.scalar.copy(stage[:, d], ps[:])

                nc.vector.tensor_scalar(out=mask, in0=stage[:], scalar1=0.5,
                                        scalar2=None, op0=mybir.AluOpType.is_gt)
                nc.vector.tensor_scalar(out=stage[:], in0=stage[:], scalar1=1.0,
                                        scalar2=None, op0=mybir.AluOpType.max)
                nc.scalar.activation(stage[:], stage[:], mybir.ActivationFunctionType.Sqrt)
                nc.vector.reciprocal(stage[:], stage[:])
                nc.vector.tensor_tensor(out=dinv[:], in0=stage[:], in1=mask,
                                        op=mybir.AluOpType.mult)

                hcur = hA
                for lay in range(cfg.layers):
                    hnxt = hB if hcur is hA else hA
                    # ---- z~ compute (node-major into stage) ----
                    for q in range(NQ):
                        ph = 0 if q < NQ // 2 else 64
                        qq = q % (NQ // 2)
                        pzt = pz.tile([64, 512], mybir.dt.float32, tag="pz")
                        nc.tensor.matmul(
                            pzt[:], wt[ph:ph + 64, lay],
                            hcur[ph:ph + 64, qq * 512:(qq + 1) * 512],
                            start=True, stop=True)
                        zb = zp.tile([64, 512], mybir.dt.float32, tag="zbt")
                        nc.vector.tensor_scalar(
                            out=zb[:], in0=pzt[:],
                            scalar1=bs[ph:ph + 64, lay:lay + 1],
                            scalar2=None, op0=mybir.AluOpType.add)
                        for j in range(4):
                            blk = 4 * q + j
                            ptt = pt.tile([128, F], mybir.dt.float32, tag="pt")
                            nc.tensor.transpose(
                                ptt[:], zb[:, j * 128:(j + 1) * 128], ident[:64, :64])
                            nc.scalar.activation(
                                stage[:, blk], ptt[:],
                                mybir.ActivationFunctionType.Copy,
                                scale=dinv[:, blk, 0:1])
                    # fp16 copy for the gather matmuls; f32 zdram for overflow
                    nc.vector.tensor_scalar(out=zb16[:], in0=stage[:], scalar1=0.0,
                                            scalar2=None, op0=mybir.AluOpType.add)
                    nc.sync.dma_start(
                        zdram[:].rearrange("(p c) f -> p c f", p=128), stage[:])

                    # ---- gather phase: per src block, one-hot matmuls -> cells
                    for sb in range(NBLK):
                        ob = obp.tile([128, SROW], mybir.dt.float16, tag="ob")
                        nc.sync.dma_start(ob[:], og_hbm[sb * 128:(sb + 1) * 128, :])
                        mstg = msp.tile([128, SCH, F], mybir.dt.float16, tag="mstg")
                        for c8 in range(0, SCH, 8):
                            w8 = min(8, SCH - c8)
                            pm = pmg.tile([128, 8, F], mybir.dt.float32, tag="pmg")
                            for j in range(w8):
                                nc.tensor.matmul(
                                    pm[:, j], ob[:, (c8 + j) * 128:(c8 + j + 1) * 128],
                                    zb16[:, sb], start=True, stop=True)
                            nc.scalar.copy(mstg[:, c8:c8 + w8], pm[:, :w8])
                        nc.sync.dma_start(
                            cells[sb * SROW:(sb + 1) * SROW, :]
                            .rearrange("(p c) f -> p c f", p=128), mstg[:])

                    # ---- scatter phase: per dest block ----
                    cells4 = cells[:].rearrange("(s d k) f -> d s k f", s=NBLK, d=NBLK)
                    for og in range(NBLK // OVG):
                        novi = OVG * OV_CB * 128
                        ovm = ovp.tile([128, OVG * OV_CB, F], mybir.dt.float32, tag="ovm")
                        nc.gpsimd.dma_gather(
                            ovm[:], zdram[:],
                            ovcolr[:, og * (novi // 16):(og + 1) * (novi // 16)],
                            novi, novi, F, single_packet=False)
                        for dd in range(OVG):
                            d = og * OVG + dd
                            # cell-major read: one 4KB cell per partition
                            msgs = mp.tile([80, CELL, F], mybir.dt.float16, tag="msgs")
                            nc.sync.dma_start(msgs[:], cells4[d])
                            ps = psc.tile([128, F], mybir.dt.float32, tag="psc")
                            oh = onehot_cell(d)
                            for k in range(CELL):
                                nc.tensor.matmul(ps[:], oh[:, :, k], msgs[:, k],
                                                 start=(k == 0), stop=False)
                            ohov = onehot_ov(d)
                            for c in range(OV_CB):
                                nc.tensor.matmul(
                                    ps[:], ohov[:, c * 128:(c + 1) * 128],
                                    ovm[:, dd * OV_CB + c],
                                    start=False, stop=(c == OV_CB - 1))
                            if lay < cfg.layers - 1:
                                hm = zp.tile([128, F], mybir.dt.float32, tag="hm")
                                nc.scalar.activation(
                                    hm[:], ps[:], mybir.ActivationFunctionType.Relu,
                                    scale=dinv[:, d, 0:1])
                                pbtt = pbt.tile([64, 128], mybir.dt.float32, tag="pbt")
                                nc.tensor.transpose(pbtt[:], hm[:], ident[:])
                                ph = 0 if d < NBLK // 2 else 64
                                bq = d % (NBLK // 2)
                                nc.scalar.copy(
                                    hnxt[ph:ph + 64, bq * 128:(bq + 1) * 128], pbtt[:])
                            else:
                                nc.scalar.activation(
                                    stage[:, d], ps[:],
                                    mybir.ActivationFunctionType.Copy,
                                    scale=dinv[:, d, 0:1])
                    hcur = hnxt
                nc.sync.dma_start(
                    out_hbm[:].rearrange("p (c f) -> p c f", c=NBLK), stage[:])

            for _ in range(trips):
                one_trip()

    nc.compile()
    return nc


def _prep_inputs(cfg: Cfg, x, edge_index, Ws, bs_):
    """Index/layout marshaling for one graph (no value arithmetic)."""
    OV_CB = cfg.OV_CB
    EPADOV = cfg.epadov
    row = np.asarray(edge_index[:, 0], np.int64)
    col = np.asarray(edge_index[:, 1], np.int64)
    d = row >> 7
    s = col >> 7
    cell = s * NBLK + d
    order = np.argsort(cell, kind="stable")
    cs = cell[order]
    counts = np.bincount(cell, minlength=NBLK * NBLK)
    starts = np.cumsum(counts) - counts
    within = np.arange(len(row)) - starts[cs]
    main = within < CELL
    ro, co, wo = row[order], col[order], within
    so, do = s[order], d[order]

    gslot = so * SROW + do * CELL + wo          # src-major grid slot
    gsrc = np.full(EPADG, 999, np.int32)
    gsrc[gslot[main]] = (co & 127)[main]
    rowcell_f = np.full(EPADG, 999.0, np.float32)
    rowcell_f[gslot[main]] = (ro & 127)[main].astype(np.float32)

    # offset-major one-hot per src block, columns permuted so gather-mm
    # chunk j partition p addresses within-sb row v = p*SCH + j
    ohg = (gsrc.reshape(NBLK, SROW)[:, None, :] ==
           np.arange(128, dtype=np.int32)[None, :, None]).astype(np.float16)
    perm = np.arange(SROW).reshape(128, SCH).transpose(1, 0).reshape(-1)
    ohg = np.ascontiguousarray(ohg[:, :, perm])

    # rowcell [128, 2560]: partition = src block s (80 used), free = d*32+k
    rowcell_t = np.full((128, SROW), 999.0, np.float16)
    rowcell_t[:NBLK] = rowcell_f.reshape(NBLK, SROW).astype(np.float16)

    # overflow edges (dest-bucketed, per-edge gather path)
    ov = ~main
    rov, cov, dov = ro[ov], co[ov], do[ov]
    ocounts = np.bincount(dov, minlength=NBLK)
    assert ocounts.max() <= OV_CB * 128, f"ov overflow: {ocounts.max()}"
    oorder = np.argsort(dov, kind="stable")
    ostarts = np.cumsum(ocounts) - ocounts
    obase = np.repeat(np.arange(NBLK) * OV_CB * 128, ocounts)
    owithin = np.arange(len(rov)) - np.repeat(ostarts, ocounts)
    oslots = obase + owithin
    ovrow = np.full(EPADOV, 999.0, np.float32)
    ovcol = np.zeros(EPADOV, np.int64)
    ovrow[oslots] = (rov & 127)[oorder]
    ovcol[oslots] = cov[oorder]
    ovcolr = ((ovcol & 127) * NBLK + (ovcol >> 7)).astype(np.int16)

    def wrap16(a):
        w = a.reshape(-1, 16).T
        return np.tile(w, (8, 1))

    ovrow_t = np.ascontiguousarray(ovrow.reshape(-1, 128).T.astype(np.float16))
    ovcolr_t = wrap16(ovcolr)

    xT = np.zeros((64, N), np.float32)
    xT[:, :NV] = np.asarray(x, np.float32).T
    x_packed = np.concatenate([xT[:, :N // 2], xT[:, N // 2:]], axis=0)

    w_t = np.zeros((128, len(Ws), F), np.float32)
    bias = np.zeros((128, len(Ws)), np.float32)
    for l, (W, b) in enumerate(zip(Ws, bs_)):
        w_t[:64, l] = np.asarray(W, np.float32).T
        w_t[64:, l] = np.asarray(W, np.float32).T
        bias[:64, l] = np.asarray(b, np.float32)
        bias[64:, l] = np.asarray(b, np.float32)

    return {
        "x_packed": x_packed,
        "w_t": np.ascontiguousarray(w_t.reshape(128, -1)),
        "bias": bias,
        "ident": np.eye(128, dtype=np.float32),
        "iota_t": np.tile(np.tile(np.arange(128, dtype=np.float16), SCH), (128, 1)),
        "iota_x": np.tile(np.repeat(np.arange(128, dtype=np.float16), CELL), (128, 1)),
        "rowcell": rowcell_t,
        "ohg": np.ascontiguousarray(ohg.reshape(NBLK * 128, SROW)),
        "ovrow": ovrow_t,
        "ovcolr": ovcolr_t,
    }


def _unpack_output(out_pm):
    o = out_pm.reshape(128, NBLK, F).transpose(1, 0, 2).reshape(N, F)
    return o[:NV]


def kernel(x, edge_index, W1, b1, W2, b2, W3, b3):
    x = np.asarray(x)
    edge_index = np.asarray(edge_index)
    Ws = [np.asarray(W1), np.asarray(W2), np.asarray(W3)]
    bs_ = [np.asarray(b1), np.asarray(b2), np.asarray(b3)]
    nb = x.shape[0]
    assert x.shape == (B, NV, F) and edge_index.shape == (B, E, 2)

    cfg = make_cfg(edge_index)
    in_maps = [_prep_inputs(cfg, x[g], edge_index[g], Ws, bs_) for g in range(nb)]
    nc = _build(cfg)
    try:
        res = run_bass_kernel_spmd(nc, in_maps, CORES).results
    except Exception:
        # transient NRT device wedge recovers on a fresh attempt
        res = run_bass_kernel_spmd(nc, in_maps, CORES).results
    out = np.stack([_unpack_output(res[g]["out_pm"]) for g in range(nb)])
    return out.astype(np.float32)


# revision 11
# speedup vs baseline: 3.6332x; 1.0658x over previous
"""Batched 3-layer GCN (nn_BatchGCN) on 8 TRN2 NeuronCores — one graph per core.

Math per graph, per layer:  h' = Ahat @ (h @ W.T + b),  Ahat = D^-1/2 A D^-1/2
(duplicate edges accumulate; relu between layers 1-2).  The symmetric
normalization factors node-wise:
    out[i] = dinv[i] * sum_{e: row_e==i} (dinv * z)[col_e],  z = h@W.T + b
so no per-edge scalar work is needed — only dense node-wise scaling.

Cell-grid device algorithm (per core).  The edge permutation (gather of
z[col] into dest-bucket order) is done as an 80x80 (src-block x dest-block)
grid transpose through DRAM with contiguous-cell DMAs — no per-edge DMA
descriptors (the old dma_gather path was descriptor-count-bound at ~10ns per
256B descriptor, 160K descriptors/layer):
  - host buckets edges into (s,d) cells of 32 slots; overflow edges (~2%)
    fall back to the per-edge dma_gather path (OV_CB chunks per dest block)
  - per layer: z~ = dinv*(h@W.T+b) node-major (PE z-matmuls + transposes,
    scale fused into the ScalarE drain), fp16 copy for the gather matmuls
  - GATHER per src block sb: host-shipped offset-major one-hot OHT[off,slot]
    (fp16, pure index data) as PE stationary x z~[sb] produces slot-major
    messages; drained fp16; one contiguous DMA writes sb's cell row
  - SCATTER per dest block d: one strided DMA reads d's 80 cells (4KB/cell,
    one per SBUF partition); is_equal one-hot [s, doff, k] built on DVE with
    packed-fp16 operands (2x fast path) + 32 accumulating matmuls contract
    over the 80 cell partitions; overflow chunk matmuls add in; relu*dinv
    fused into the PSUM drain
  - degree pass reuses the same one-hot structure against a ones rhs
Messages travel fp16 (~1e-4 rel err vs f32 reference; tolerance 2e-2).

Host-side work is index/layout marshaling only (transpose/pad/bucket the
given index arrays and expand index one-hots); all arithmetic on tensor
values happens on-device.
"""
from dataclasses import dataclass

import numpy as np

import concourse.bacc as bacc
import concourse.mybir as mybir
import concourse.tile as tile
from concourse.bass import broadcast_tensor_aps
from concourse.bass_utils import run_bass_kernel_spmd
from concourse.library_config import mlp

B, NV, E, F = 8, 10000, 160000, 64
N = 10240
NBLK = 80
CELL = 32
SROW = NBLK * CELL          # 2560 slots per src-block row (= per dest block)
SCH = SROW // 128           # 20 gather-matmul chunks per src block
EPADG = NBLK * SROW         # 204800 grid slots
CORES = list(range(8))


@dataclass(frozen=True)
class Cfg:
    OV_CB: int              # overflow chunks (of 128 slots) per dest block
    OVG: int = 8            # dest blocks per overflow gather call
    layers: int = 3

    @property
    def epadov(self):
        return NBLK * self.OV_CB * 128


def make_cfg(edge_index):
    mx = 0
    for g in range(edge_index.shape[0]):
        row = np.asarray(edge_index[g, :, 0], np.int64)
        col = np.asarray(edge_index[g, :, 1], np.int64)
        cell = (col >> 7) * NBLK + (row >> 7)
        ccnt = np.bincount(cell, minlength=NBLK * NBLK)
        ovper = np.maximum(ccnt - CELL, 0).reshape(NBLK, NBLK).sum(axis=0)
        mx = max(mx, int(ovper.max()))
    return Cfg(OV_CB=max(1, -(-mx // 128)))


def _build(cfg: Cfg, trips: int = 1):
    OV_CB = cfg.OV_CB
    EPADOV = cfg.epadov
    OVG = cfg.OVG
    NQ = N // 512

    nc = bacc.Bacc("TRN2", debug=False)
    x_hbm = nc.dram_tensor("x_packed", [128, N // 2], mybir.dt.float32, kind="ExternalInput")
    w_hbm = nc.dram_tensor("w_t", [128, cfg.layers * F], mybir.dt.float32, kind="ExternalInput")
    b_hbm = nc.dram_tensor("bias", [128, cfg.layers], mybir.dt.float32, kind="ExternalInput")
    i_hbm = nc.dram_tensor("ident", [128, 128], mybir.dt.float32, kind="ExternalInput")
    t_hbm = nc.dram_tensor("iota_t", [128, SROW], mybir.dt.float16, kind="ExternalInput")
    tx_hbm = nc.dram_tensor("iota_x", [128, 128 * 2 * CELL], mybir.dt.float16, kind="ExternalInput")
    rc_hbm = nc.dram_tensor("rowcell", [128, SROW], mybir.dt.float16, kind="ExternalInput")
    og_hbm = nc.dram_tensor("ohg", [NBLK * 128, SROW], mybir.dt.float16, kind="ExternalInput")
    ovr_hbm = nc.dram_tensor("ovrow", [128, EPADOV // 128], mybir.dt.float16, kind="ExternalInput")
    ovc_hbm = nc.dram_tensor("ovcolr", [128, EPADOV // 16], mybir.dt.int16, kind="ExternalInput")
    out_hbm = nc.dram_tensor("out_pm", [128, NBLK * F], mybir.dt.float32, kind="ExternalOutput")
    zdram = nc.dram_tensor("zdram", [N, F], mybir.dt.float32)
    cells = nc.dram_tensor("cells", [EPADG, F], mybir.dt.float16)

    with tile.TileContext(nc) as tc:
        with (
            tc.tile_pool(name="const", bufs=1) as cp,
            tc.tile_pool(name="state", bufs=1) as sp,
            tc.tile_pool(name="ohgb", bufs=2) as obp,
            tc.tile_pool(name="oh", bufs=2) as ohp,
            tc.tile_pool(name="mst", bufs=2) as msp,
            tc.tile_pool(name="msg", bufs=3) as mp,
            tc.tile_pool(name="ovm", bufs=2) as ovp,
            tc.tile_pool(name="zb", bufs=2) as zp,
            tc.tile_pool(name="pz", bufs=1, space="PSUM") as pz,
            tc.tile_pool(name="pt", bufs=2, space="PSUM") as pt,
            tc.tile_pool(name="pmg", bufs=2, space="PSUM") as pmg,
            tc.tile_pool(name="psc", bufs=2, space="PSUM") as psc,
            tc.tile_pool(name="pbt", bufs=1, space="PSUM") as pbt,
        ):
            nc.gpsimd.load_library(mlp)

            wt = cp.tile([128, cfg.layers, F], mybir.dt.float32)
            nc.sync.dma_start(wt[:], w_hbm[:].rearrange("p (l f) -> p l f", l=cfg.layers))
            bs = cp.tile([128, cfg.layers], mybir.dt.float32)
            nc.sync.dma_start(bs[:], b_hbm[:])
            ident = cp.tile([128, 128], mybir.dt.float32)
            nc.sync.dma_start(ident[:], i_hbm[:])
            iota = cp.tile([128, SROW], mybir.dt.float16)
            nc.sync.dma_start(iota[:], t_hbm[:])
            iotax = cp.tile([128, 128 * 2 * CELL], mybir.dt.float16)
            nc.sync.dma_start(iotax[:], tx_hbm[:])
            rowcell = cp.tile([128, SROW], mybir.dt.float16)
            nc.sync.dma_start(rowcell[:], rc_hbm[:])
            ovrow = cp.tile([128, EPADOV // 128], mybir.dt.float16)
            nc.sync.dma_start(ovrow[:], ovr_hbm[:])
            ovcolr = cp.tile([128, EPADOV // 16], mybir.dt.int16)
            nc.sync.dma_start(ovcolr[:], ovc_hbm[:])
            ones_b = cp.tile([128, F], mybir.dt.float16)
            nc.vector.memset(ones_b[:], 1.0)
            ones_f = cp.tile([128, F], mybir.dt.float32)
            nc.vector.memset(ones_f[:], 1.0)

            hA = sp.tile([128, N // 2], mybir.dt.float32, tag="hA")
            hB = sp.tile([128, N // 2], mybir.dt.float32, tag="hB")
            dinv = sp.tile([128, NBLK, F], mybir.dt.float32, tag="dinv")
            stage = sp.tile([128, NBLK, F], mybir.dt.float32, tag="stage")
            zb16 = sp.tile([128, NBLK, F], mybir.dt.float16, tag="zb16")
            mask = hB[:].rearrange("p (c f) -> p c f", c=NBLK)

            def onehot_cell2(dp):
                # cell-major scatter one-hot for a PAIR of dest blocks,
                # [s, doff, k2] layout (k2<32: block 2dp, k2>=32: 2dp+1);
                # packed 2-byte last dims keep the DVE 2x fast path
                oh = ohp.tile([80, 128, 2 * CELL], mybir.dt.float16, tag="oh")
                ro3 = rowcell[:80, dp * 2 * CELL:(dp + 1) * 2 * CELL] \
                    .rearrange("p k -> p () k")
                io3 = iotax[:80].rearrange("p (j k) -> p j k", j=128)
                a, bb = broadcast_tensor_aps(io3, ro3)
                nc.vector.tensor_tensor(out=oh[:], in0=a, in1=bb,
                                        op=mybir.AluOpType.is_equal)
                return oh

            def onehot_ov(d):
                oh = ohp.tile([128, OV_CB * 128], mybir.dt.float32, tag="ohov")
                ro3 = ovrow[:, d * OV_CB:(d + 1) * OV_CB].rearrange("p c -> p c ()")
                io3 = iota[:, :OV_CB * 128].rearrange("p (c k) -> p c k", c=OV_CB)
                a, bb = broadcast_tensor_aps(io3, ro3)
                nc.vector.tensor_tensor(
                    out=oh[:].rearrange("p (c k) -> p c k", c=OV_CB),
                    in0=a, in1=bb, op=mybir.AluOpType.is_equal)
                return oh

            def one_trip():
                nc.sync.dma_start(hA[:], x_hbm[:])
                # ---- degree pass ----
                for dp in range(NBLK // 2):
                    oh = onehot_cell2(dp)
                    for half in range(2):
                        d = 2 * dp + half
                        ps = psc.tile([128, F], mybir.dt.float32, tag="psc")
                        for k in range(CELL):
                            nc.tensor.matmul(ps[:], oh[:, :, half * CELL + k],
                                             ones_b[:80], start=(k == 0), stop=False)
                        ohov = onehot_ov(d)
                        for c in range(OV_CB):
                            nc.tensor.matmul(ps[:], ohov[:, c * 128:(c + 1) * 128],
                                             ones_f[:], start=False,
                                             stop=(c == OV_CB - 1))
                        nc.scalar.copy(stage[:, d], ps[:])

                nc.vector.tensor_scalar(out=mask, in0=stage[:], scalar1=0.5,
                                        scalar2=None, op0=mybir.AluOpType.is_gt)
                nc.vector.tensor_scalar(out=stage[:], in0=stage[:], scalar1=1.0,
                                        scalar2=None, op0=mybir.AluOpType.max)
                nc.scalar.activation(stage[:], stage[:], mybir.ActivationFunctionType.Sqrt)
                nc.vector.reciprocal(stage[:], stage[:])
                nc.vector.tensor_tensor(out=dinv[:], in0=stage[:], in1=mask,
                                        op=mybir.AluOpType.mult)

                hcur = hA
                for lay in range(cfg.layers):
                    hnxt = hB if hcur is hA else hA
                    # ---- z~ compute (node-major into stage) ----
                    for q in range(NQ):
                        ph = 0 if q < NQ // 2 else 64
                        qq = q % (NQ // 2)
                        pzt = pz.tile([64, 512], mybir.dt.float32, tag="pz")
                        nc.tensor.matmul(
                            pzt[:], wt[ph:ph + 64, lay],
                            hcur[ph:ph + 64, qq * 512:(qq + 1) * 512],
                            start=True, stop=True)
                        zb = zp.tile([64, 512], mybir.dt.float32, tag="zbt")
                        nc.vector.tensor_scalar(
                            out=zb[:], in0=pzt[:],
                            scalar1=bs[ph:ph + 64, lay:lay + 1],
                            scalar2=None, op0=mybir.AluOpType.add)
                        for j in range(4):
                            blk = 4 * q + j
                            ptt = pt.tile([128, F], mybir.dt.float32, tag="pt")
                            nc.tensor.transpose(
                                ptt[:], zb[:, j * 128:(j + 1) * 128], ident[:64, :64])
                            nc.scalar.activation(
                                stage[:, blk], ptt[:],
                                mybir.ActivationFunctionType.Copy,
                                scale=dinv[:, blk, 0:1])
                    # fp16 copy for the gather matmuls; f32 zdram for overflow
                    nc.vector.tensor_scalar(out=zb16[:], in0=stage[:], scalar1=0.0,
                                            scalar2=None, op0=mybir.AluOpType.add)
                    nc.sync.dma_start(
                        zdram[:].rearrange("(p c) f -> p c f", p=128), stage[:])

                    # ---- gather phase: per src block, one-hot matmuls -> cells
                    for sb in range(NBLK):
                        ob = obp.tile([128, SROW], mybir.dt.float16, tag="ob")
                        nc.sync.dma_start(ob[:], og_hbm[sb * 128:(sb + 1) * 128, :])
                        mstg = msp.tile([128, SCH, F], mybir.dt.float16, tag="mstg")
                        for c8 in range(0, SCH, 8):
                            w8 = min(8, SCH - c8)
                            pm = pmg.tile([128, 8, F], mybir.dt.float32, tag="pmg")
                            for j in range(w8):
                                nc.tensor.matmul(
                                    pm[:, j], ob[:, (c8 + j) * 128:(c8 + j + 1) * 128],
                                    zb16[:, sb], start=True, stop=True)
                            nc.scalar.copy(mstg[:, c8:c8 + w8], pm[:, :w8])
                        nc.sync.dma_start(
                            cells[sb * SROW:(sb + 1) * SROW, :]
                            .rearrange("(p c) f -> p c f", p=128), mstg[:])

                    # ---- scatter phase: per dest block ----
                    cells4 = cells[:].rearrange("(s dp k2) f -> dp s k2 f",
                                                s=NBLK, dp=NBLK // 2)
                    for og in range(NBLK // OVG):
                        novi = OVG * OV_CB * 128
                        ovm = ovp.tile([128, OVG * OV_CB, F], mybir.dt.float32, tag="ovm")
                        nc.gpsimd.dma_gather(
                            ovm[:], zdram[:],
                            ovcolr[:, og * (novi // 16):(og + 1) * (novi // 16)],
                            novi, novi, F, single_packet=False)
                        for dd in range(0, OVG, 2):
                            dp = (og * OVG + dd) // 2
                            # paired cell-major read: 8KB per partition
                            msgs = mp.tile([80, 2 * CELL, F], mybir.dt.float16,
                                           tag="msgs")
                            nc.sync.dma_start(msgs[:], cells4[dp])
                            oh = onehot_cell2(dp)
                            for half in range(2):
                                d = 2 * dp + half
                                ps = psc.tile([128, F], mybir.dt.float32, tag="psc")
                                for k in range(CELL):
                                    nc.tensor.matmul(
                                        ps[:], oh[:, :, half * CELL + k],
                                        msgs[:, half * CELL + k],
                                        start=(k == 0), stop=False)
                                ohov = onehot_ov(d)
                                for c in range(OV_CB):
                                    nc.tensor.matmul(
                                        ps[:], ohov[:, c * 128:(c + 1) * 128],
                                        ovm[:, (dd + half) * OV_CB + c],
                                        start=False, stop=(c == OV_CB - 1))
                                if lay < cfg.layers - 1:
                                    hm = zp.tile([128, F], mybir.dt.float32, tag="hm")
                                    nc.scalar.activation(
                                        hm[:], ps[:], mybir.ActivationFunctionType.Relu,
                                        scale=dinv[:, d, 0:1])
                                    pbtt = pbt.tile([64, 128], mybir.dt.float32,
                                                    tag="pbt")
                                    nc.tensor.transpose(pbtt[:], hm[:], ident[:])
                                    ph = 0 if d < NBLK // 2 else 64
                                    bq = d % (NBLK // 2)
                                    nc.scalar.copy(
                                        hnxt[ph:ph + 64, bq * 128:(bq + 1) * 128],
                                        pbtt[:])
                                else:
                                    nc.scalar.activation(
                                        stage[:, d], ps[:],
                                        mybir.ActivationFunctionType.Copy,
                                        scale=dinv[:, d, 0:1])
                    hcur = hnxt
                nc.sync.dma_start(
                    out_hbm[:].rearrange("p (c f) -> p c f", c=NBLK), stage[:])

            for _ in range(trips):
                one_trip()

    nc.compile()
    return nc


def _prep_inputs(cfg: Cfg, x, edge_index, Ws, bs_):
    """Index/layout marshaling for one graph (no value arithmetic)."""
    OV_CB = cfg.OV_CB
    EPADOV = cfg.epadov
    row = np.asarray(edge_index[:, 0], np.int64)
    col = np.asarray(edge_index[:, 1], np.int64)
    d = row >> 7
    s = col >> 7
    cell = s * NBLK + d
    order = np.argsort(cell, kind="stable")
    cs = cell[order]
    counts = np.bincount(cell, minlength=NBLK * NBLK)
    starts = np.cumsum(counts) - counts
    within = np.arange(len(row)) - starts[cs]
    main = within < CELL
    ro, co, wo = row[order], col[order], within
    so, do = s[order], d[order]

    gslot = so * SROW + do * CELL + wo          # src-major grid slot
    gsrc = np.full(EPADG, 999, np.int32)
    gsrc[gslot[main]] = (co & 127)[main]
    rowcell_f = np.full(EPADG, 999.0, np.float32)
    rowcell_f[gslot[main]] = (ro & 127)[main].astype(np.float32)

    # offset-major one-hot per src block, columns permuted so gather-mm
    # chunk j partition p addresses within-sb row v = p*SCH + j
    ohg = (gsrc.reshape(NBLK, SROW)[:, None, :] ==
           np.arange(128, dtype=np.int32)[None, :, None]).astype(np.float16)
    perm = np.arange(SROW).reshape(128, SCH).transpose(1, 0).reshape(-1)
    ohg = np.ascontiguousarray(ohg[:, :, perm])

    # rowcell [128, 2560]: partition = src block s (80 used), free = d*32+k
    rowcell_t = np.full((128, SROW), 999.0, np.float16)
    rowcell_t[:NBLK] = rowcell_f.reshape(NBLK, SROW).astype(np.float16)

    # overflow edges (dest-bucketed, per-edge gather path)
    ov = ~main
    rov, cov, dov = ro[ov], co[ov], do[ov]
    ocounts = np.bincount(dov, minlength=NBLK)
    assert ocounts.max() <= OV_CB * 128, f"ov overflow: {ocounts.max()}"
    oorder = np.argsort(dov, kind="stable")
    ostarts = np.cumsum(ocounts) - ocounts
    obase = np.repeat(np.arange(NBLK) * OV_CB * 128, ocounts)
    owithin = np.arange(len(rov)) - np.repeat(ostarts, ocounts)
    oslots = obase + owithin
    ovrow = np.full(EPADOV, 999.0, np.float32)
    ovcol = np.zeros(EPADOV, np.int64)
    ovrow[oslots] = (rov & 127)[oorder]
    ovcol[oslots] = cov[oorder]
    ovcolr = ((ovcol & 127) * NBLK + (ovcol >> 7)).astype(np.int16)

    def wrap16(a):
        w = a.reshape(-1, 16).T
        return np.tile(w, (8, 1))

    ovrow_t = np.ascontiguousarray(ovrow.reshape(-1, 128).T.astype(np.float16))
    ovcolr_t = wrap16(ovcolr)

    xT = np.zeros((64, N), np.float32)
    xT[:, :NV] = np.asarray(x, np.float32).T
    x_packed = np.concatenate([xT[:, :N // 2], xT[:, N // 2:]], axis=0)

    w_t = np.zeros((128, len(Ws), F), np.float32)
    bias = np.zeros((128, len(Ws)), np.float32)
    for l, (W, b) in enumerate(zip(Ws, bs_)):
        w_t[:64, l] = np.asarray(W, np.float32).T
        w_t[64:, l] = np.asarray(W, np.float32).T
        bias[:64, l] = np.asarray(b, np.float32)
        bias[64:, l] = np.asarray(b, np.float32)

    return {
        "x_packed": x_packed,
        "w_t": np.ascontiguousarray(w_t.reshape(128, -1)),
        "bias": bias,
        "ident": np.eye(128, dtype=np.float32),
        "iota_t": np.tile(np.tile(np.arange(128, dtype=np.float16), SCH), (128, 1)),
        "iota_x": np.tile(np.repeat(np.arange(128, dtype=np.float16), 2 * CELL), (128, 1)),
        "rowcell": rowcell_t,
        "ohg": np.ascontiguousarray(ohg.reshape(NBLK * 128, SROW)),
        "ovrow": ovrow_t,
        "ovcolr": ovcolr_t,
    }


def _unpack_output(out_pm):
    o = out_pm.reshape(128, NBLK, F).transpose(1, 0, 2).reshape(N, F)
    return o[:NV]


def kernel(x, edge_index, W1, b1, W2, b2, W3, b3):
    x = np.asarray(x)
    edge_index = np.asarray(edge_index)
    Ws = [np.asarray(W1), np.asarray(W2), np.asarray(W3)]
    bs_ = [np.asarray(b1), np.asarray(b2), np.asarray(b3)]
    nb = x.shape[0]
    assert x.shape == (B, NV, F) and edge_index.shape == (B, E, 2)

    cfg = make_cfg(edge_index)
    in_maps = [_prep_inputs(cfg, x[g], edge_index[g], Ws, bs_) for g in range(nb)]
    nc = _build(cfg)
    try:
        res = run_bass_kernel_spmd(nc, in_maps, CORES).results
    except Exception:
        # transient NRT device wedge recovers on a fresh attempt
        res = run_bass_kernel_spmd(nc, in_maps, CORES).results
    out = np.stack([_unpack_output(res[g]["out_pm"]) for g in range(nb)])
    return out.astype(np.float32)
